# revision 1
# baseline (speedup 1.0000x reference)
"""Trainium2 Bass kernel for single-head self-attention over image tokens.

Reference computation (per batch element b of 4):
    xf   = x[b] viewed as [N=4096 tokens, C=256]          (x stored [C, H*W] = xf.T)
    qkv  = xf @ w_qkv.T                                   -> q, k, v each [N, 512]
    sim  = (q * 64**-0.5) @ k.T                           [N, N]
    attn = softmax(sim, axis=-1)
    out  = (attn @ v) @ w_out.T + b_out + xf              [N, C]

Sharding: 8 cores = 4 batches x 2 query-row halves (2048 rows each). Each core
computes k/v for its full batch but q/out only for its half. No collectives.
Each core's x is host-rotated so its query half is always columns 0:2048
(softmax over keys is permutation invariant, so key order doesn't matter).

Matmul operands use float32r: fp32 with the mantissa rounded to 11 bits
(round-half-even on the low 12 bits, same bit layout as fp32), which streams
1 PE column/cycle instead of 4 for plain fp32. x and the weights are
pre-rounded on the host and DMAed straight into float32r tiles; on-chip
intermediates (qT/kT/v/pT) get rounded by the PSUM->SBUF copy or activation
that produces them.

On-chip layout keeps everything in the "transposed activation" orientation so
no PE transposes are needed:
    qT [512, 2048] and kT [512, N] come straight out of the QKV projection
    (x's HBM layout [C, N] is already the rhs/lhsT the PE wants);
    v [N, 512] comes from the same projection with x slices as the stationary
    operand. simT [j, i] = kT.T @ qT, pT = exp(0.125*simT), then
    outT [d, i] += v_j.T @ pT accumulates in PSUM per 1024-column j-superblock
    and the softmax denominator l[1, i] += ones.T @ (pT pairs summed on
    GpSimd). Normalization is folded in at the end of the last superblock,
    per query slice: recip(l) via a fast Newton iteration on the DVE after a
    K=1 rank-1 broadcast matmul, multiplied into the final projection output.
"""

import hashlib
import os
import shutil

import numpy as np

import concourse.bacc as bacc
import concourse.tile as tile
import concourse.mybir as mybir
from concourse.bass_utils import run_bass_kernel_spmd


def _install_neff_cache():
    """Disk-cache walrus NEFF compiles keyed on the BIR content hash.

    The axon PJRT path recompiles the NEFF in every fresh process (~minutes);
    the build here is deterministic, so identical BIR -> identical NEFF.
    """
    try:
        import concourse.bass2jax as bass2jax
        orig = bass2jax.compile_bir_kernel
        if getattr(orig, "_neff_cache_wrapped", False):
            return
        cache_dir = os.path.expanduser("~/.neuron-compile-cache/bass-neff")

        def cached(bir_json, tmpdir, neff_name="file.neff"):
            try:
                key = hashlib.sha256(
                    bir_json if isinstance(bir_json, bytes)
                    else bir_json.encode()).hexdigest()
                hit = os.path.join(cache_dir, key + ".neff")
                dst = os.path.join(tmpdir, neff_name)
                if os.path.exists(hit):
                    shutil.copyfile(hit, dst)
                    return dst
                neff = orig(bir_json, tmpdir, neff_name=neff_name)
                os.makedirs(cache_dir, exist_ok=True)
                tmp = hit + ".tmp%d" % os.getpid()
                shutil.copyfile(neff, tmp)
                os.replace(tmp, hit)
                return neff
            except Exception:
                return orig(bir_json, tmpdir, neff_name=neff_name)

        cached._neff_cache_wrapped = True
        bass2jax.compile_bir_kernel = cached
    except Exception:
        pass


_install_neff_cache()

F32 = mybir.dt.float32
F32R = mybir.dt.float32r
Exp = mybir.ActivationFunctionType.Exp

B = 4
C = 256          # model dim (2 chunks of 128)
N = 4096         # tokens per batch (64*64)
HALF = N // 2    # query rows per core
INNER = 512      # qkv inner dim (4 chunks of 128)
SCALE = 0.125    # 64 ** -0.5

NCORES = 8
NJB = 4          # j superblocks per batch
JBW = N // NJB   # 1024 key columns per superblock
NSL = 4          # i slices per core
SW = HALF // NSL # 512 query columns per slice


def build_nc(n=N, njb=NJB, nsl=NSL):
    half = n // 2
    jbw = n // njb
    assert half % SW == 0 and jbw % SW == 0 and jbw % 256 == 0
    nc = bacc.Bacc(None)
    x_r = nc.declare_dram_parameter("x_r", [C, n], F32R, isOutput=False)
    xq_f = nc.declare_dram_parameter("xq_f", [C, half], F32, isOutput=False)
    wqkvT = nc.declare_dram_parameter("wqkvT", [C, 3 * INNER], F32R, isOutput=False)
    woutT = nc.declare_dram_parameter("woutT", [INNER, C], F32R, isOutput=False)
    bout = nc.declare_dram_parameter("bout", [2, 128, 1], F32, isOutput=False)
    out = nc.declare_dram_parameter("out", [C, half], F32, isOutput=True)

    mm = nc.tensor.matmul

    with tile.TileContext(nc) as tc:
        with tc.tile_pool(name="const", bufs=1) as const, \
             tc.tile_pool(name="stream", bufs=1) as stream, \
             tc.tile_pool(name="work", bufs=2) as work, \
             tc.tile_pool(name="pp", bufs=1, space="PSUM") as pp:

            # ---- resident weights: direct f32r DMA (host pre-rounded) ----
            wq = []
            for cc in range(2):
                t = const.tile([128, 3 * INNER], F32R, tag=f"wq{cc}", name=f"wq{cc}")
                nc.sync.dma_start(t, wqkvT[cc * 128:(cc + 1) * 128, :])
                wq.append(t)

            def xchunk(cc, col, width):
                """x chunk [128, width] in f32r, shares slots with xjb tiles."""
                t = stream.tile([128, width], F32R, tag=f"xjb{cc}", bufs=2,
                                name=f"xjb{cc}", padded_shape=[128, jbw])
                nc.sync.dma_start(t, x_r[cc * 128:(cc + 1) * 128, col:col + width])
                return t

            qT = [const.tile([128, half], F32R, tag=f"qt{d}", name=f"qt{d}")
                  for d in range(4)]
            ot = [const.tile([128, half], F32, tag=f"ot{d}", name=f"ot{d}")
                  for d in range(4)]
            l_sb = const.tile([1, half], F32, tag="l_sb", name="l_sb")

            ones_col_f = const.tile([128, 1], F32, tag="ones_col_f", name="ones_col_f")
            nc.vector.memset(ones_col_f, 1.0)
            ones_col = const.tile([128, 1], F32R, tag="ones_col", name="ones_col")
            nc.vector.tensor_copy(ones_col, ones_col_f)
            ones_row_f = const.tile([1, 128], F32, tag="ones_row_f", name="ones_row_f")
            nc.vector.memset(ones_row_f, 1.0)
            ones_row = const.tile([1, 128], F32R, tag="ones_row", name="ones_row")
            nc.vector.tensor_copy(ones_row, ones_row_f)

            # ---- qT production from x columns 0:half ----
            wo = []
            xqt = []
            bt = []
            qcw = min(1024, half)
            for qch in range(half // qcw):
                xch = [xchunk(cc, qch * qcw, qcw) for cc in range(2)]
                for d in range(4):
                    for nb in range(qcw // SW):
                        ns = qch * (qcw // SW) + nb
                        ps = pp.tile([128, SW], F32, tag="sim", bufs=3, name="ps_q")
                        for cc in range(2):
                            mm(ps, wq[cc][:, d * 128:(d + 1) * 128],
                               xch[cc][:, nb * SW:(nb + 1) * SW],
                               start=(cc == 0), stop=(cc == 1))
                        nc.scalar.copy(qT[d][:, ns * SW:(ns + 1) * SW], ps)
            # final-phase constants, off the startup critical path
            # (vector-queue DMAs so the sync queue stays free for x chunks)
            for d in range(4):
                t = const.tile([128, C], F32R, tag=f"wo{d}", name=f"wo{d}")
                nc.scalar.dma_start(t, woutT[d * 128:(d + 1) * 128, :])
                wo.append(t)
            for cc in range(2):
                t = const.tile([128, half], F32, tag=f"xq{cc}", name=f"xq{cc}")
                nc.scalar.dma_start(t, xq_f[cc * 128:(cc + 1) * 128, :])
                xqt.append(t)
            for cc in range(2):
                t = const.tile([128, 1], F32, tag=f"b{cc}", name=f"b{cc}")
                nc.scalar.dma_start(t, bout[cc])
                bt.append(t)

            # residual-with-bias: xqt <- xqt + b
            for cc in range(2):
                nc.vector.tensor_scalar_add(xqt[cc], xqt[cc], bt[cc])

            # ---- attention over j superblocks ----
            deferred = []  # denominator work deferred into later PE streams
            for jb in range(njb):
                xjb = [xchunk(cc, jb * jbw, jbw) for cc in range(2)]
                # kT for this superblock: [512, jbw]
                kt = [stream.tile([128, jbw], F32R, tag=f"kt{d}", bufs=1,
                                  name=f"kt{d}") for d in range(4)]
                for d in range(4):
                    for nb in range(jbw // SW):
                        ps = pp.tile([128, SW], F32, tag="sim", bufs=3, name="ps_k")
                        for cc in range(2):
                            mm(ps, wq[cc][:, INNER + d * 128:INNER + (d + 1) * 128],
                               xjb[cc][:, nb * SW:(nb + 1) * SW],
                               start=(cc == 0), stop=(cc == 1))
                        nc.scalar.copy(kt[d][:, nb * SW:(nb + 1) * SW], ps)
                # v for this superblock: [jbw, 512] (token rows on partitions)
                vt = []
                for nj in range(jbw // 128):
                    t = stream.tile([128, INNER], F32R, tag=f"vt{nj}", bufs=1,
                                    name=f"vt{nj}")
                    ps = pp.tile([128, INNER], F32, tag="sim", bufs=3, name="ps_v")
                    for cc in range(2):
                        mm(ps, xjb[cc][:, nj * 128:(nj + 1) * 128],
                           wq[cc][:, 2 * INNER:3 * INNER],
                           start=(cc == 0), stop=(cc == 1))
                    nc.scalar.copy(t, ps)
                    vt.append(t)

                for fn in deferred:
                    fn()
                deferred.clear()

                nj8 = jbw // 128
                for s in range(nsl):
                    sl = slice(s * SW, (s + 1) * SW)
                    po = [pp.tile([128, SW], F32, tag=f"po{d}", bufs=1,
                                  name=f"po{d}") for d in range(4)]
                    pl = pp.tile([1, SW], F32, tag="aux", bufs=1, name="pl")
                    pts = []

                    sums = []  # binary tree of pT partial sums (DVE)

                    def tree_add(t):
                        sums.append([t, 0])
                        while len(sums) >= 2 and sums[-1][1] == sums[-2][1]:
                            a, lv = sums.pop()
                            b, _ = sums.pop()
                            t2 = work.tile([128, SW], F32R, tag="pt2", bufs=4,
                                           name="pt2")
                            nc.vector.tensor_add(t2, b, a)
                            sums.append([t2, lv + 1])

                    last_jb = jb == njb - 1

                    def l_update(jb=jb, sl=sl, pl=pl):
                        if jb == 0:
                            nc.vector.tensor_copy(l_sb[:, sl], pl)
                        else:
                            nc.vector.tensor_add(l_sb[:, sl], l_sb[:, sl], pl)

                    def drain_j8(j8):
                        # outT + denominator work for chunk j8 (emitted one
                        # chunk late so the PE never waits on the exp)
                        pt = pts[j8]
                        for d in range(4):
                            mm(po[d], vt[j8][:, d * 128:(d + 1) * 128], pt,
                               start=(j8 == 0), stop=(j8 == nj8 - 1))
                        if last_jb:
                            # inline pT pair sums: the finalize below needs l
                            # with no deferral room
                            if j8 % 2 == 1:
                                pt2 = work.tile([128, SW], F32R, tag="pt2",
                                                bufs=4, name="pt2")
                                nc.vector.tensor_add(pt2, pts[j8 - 1], pt)
                                mm(pl, ones_col, pt2,
                                   start=(j8 == 1), stop=(j8 == nj8 - 1))
                        else:
                            # tree-sum the pT chunks on the DVE; the single
                            # ones-matmul + l update are deferred into a later
                            # PE stream so the PE never waits on the adder tree
                            tree_add(pt)
                            if j8 == nj8 - 1:
                                assert len(sums) == 1
                                pt8 = sums[0][0]

                                def flush(pl=pl, pt8=pt8, upd=l_update):
                                    mm(pl, ones_col, pt8, start=True, stop=True)
                                    upd()
                                deferred.append(flush)

                    for j8 in range(nj8):
                        ps = pp.tile([128, SW], F32, tag="sim", bufs=3, name="ps_s")
                        for d in range(4):
                            mm(ps, kt[d][:, j8 * 128:(j8 + 1) * 128], qT[d][:, sl],
                               start=(d == 0), stop=(d == 3))
                        pt = work.tile([128, SW], F32R, tag="pt", bufs=4, name="pt")
                        nc.scalar.activation(pt, ps, Exp, scale=SCALE)
                        pts.append(pt)
                        if j8 > 0:
                            drain_j8(j8 - 1)
                        if j8 == 2:
                            for fn in deferred:
                                fn()
                            deferred.clear()
                    drain_j8(nj8 - 1)
                    if last_jb:
                        l_update()
                    for d in range(4):
                        if jb == 0:
                            nc.vector.tensor_copy(ot[d][:, sl], po[d])
                        else:
                            nc.vector.tensor_add(ot[d][:, sl], ot[d][:, sl], po[d])

                    if jb == njb - 1:
                        # ---- finalize slice s: normalize + project + out ----
                        l_rs = work.tile([1, SW], F32R, tag="l_rs", bufs=2,
                                         name="l_rs")
                        nc.scalar.copy(l_rs, l_sb[:, sl])
                        pb = pp.tile([128, SW], F32, tag="sim", bufs=3, name="pb")
                        mm(pb, ones_row, l_rs, start=True, stop=True)
                        bc = work.tile([128, SW], F32, tag="bc", bufs=2, name="bc")
                        rsc = work.tile([128, SW], F32, tag="rsc", bufs=2,
                                        name="rsc")
                        nc.vector.reciprocal_approx_accurate(bc, pb, rsc)
                        otr = [work.tile([128, SW], F32R, tag=f"otr{d}", bufs=1,
                                         name=f"otr{d}") for d in range(4)]
                        for d in range(4):
                            nc.scalar.copy(otr[d], ot[d][:, sl])
                        for cc in range(2):
                            pf = pp.tile([128, SW], F32, tag="sim", bufs=3,
                                         name="pf")
                            for d in range(4):
                                mm(pf, wo[d][:, cc * 128:(cc + 1) * 128], otr[d],
                                   start=(d == 0), stop=(d == 3))
                            fo = work.tile([128, SW], F32, tag="fo", bufs=2,
                                           name="fo")
                            nc.vector.tensor_mul(fo, pf, bc)
                            nc.vector.tensor_add(fo, fo, xqt[cc][:, sl])
                            nc.sync.dma_start(out[cc * 128:(cc + 1) * 128, sl], fo)

    nc.finalize()
    return nc


_NC_CACHE = None


def _get_nc():
    global _NC_CACHE
    if _NC_CACHE is None:
        _NC_CACHE = build_nc()
    return _NC_CACHE


def _round_f32r(a):
    """fp32 -> float32r rounding (round-half-even on the low 12 mantissa
    bits), matching the hardware's fp32_to_fp32r conversion."""
    bits = np.ascontiguousarray(a, dtype=np.float32).view(np.uint32)
    rem = bits & np.uint32(0xFFF)
    base = bits & np.uint32(0xFFFFF000)
    up = (rem > 0x800) | ((rem == 0x800) & (((bits >> np.uint32(12)) & np.uint32(1)) == 1))
    return (base + np.where(up, np.uint32(0x1000), np.uint32(0))).view(np.float32)


def prepare_in_maps(x, w_qkv, w_out, b_out):
    x = np.asarray(x, dtype=np.float32)
    w_qkv = np.asarray(w_qkv, dtype=np.float32)
    w_out = np.asarray(w_out, dtype=np.float32)
    b_out = np.asarray(b_out, dtype=np.float32)

    xr = x.reshape(B, C, N)
    wqkvT = _round_f32r(np.ascontiguousarray(w_qkv.T))   # [C, 1536]
    woutT = _round_f32r(np.ascontiguousarray(w_out.T))   # [512, C]
    bout = np.ascontiguousarray(b_out.reshape(2, 128, 1))

    in_maps = []
    for c in range(NCORES):
        b, h = divmod(c, 2)
        if h == 0:
            x_rot = xr[b]
        else:  # rotate so this core's query half sits in columns 0:HALF
            x_rot = np.concatenate([xr[b][:, HALF:], xr[b][:, :HALF]], axis=1)
        in_maps.append({
            "x_r": _round_f32r(x_rot),
            "xq_f": np.ascontiguousarray(x_rot[:, :HALF]),
            "wqkvT": wqkvT,
            "woutT": woutT,
            "bout": bout,
        })
    return in_maps


def postprocess(results):
    outs = [results[c]["out"] for c in range(NCORES)]
    full = np.stack([np.concatenate([outs[2 * b], outs[2 * b + 1]], axis=1)
                     for b in range(B)])               # [B, C, N]
    return full.reshape(B, C, 64, 64).astype(np.float32)


def kernel(x, w_qkv, w_out, b_out):
    in_maps = prepare_in_maps(x, w_qkv, w_out, b_out)
    res = run_bass_kernel_spmd(_get_nc(), in_maps, core_ids=list(range(NCORES)))
    return postprocess(res.results)



# revision 6
# speedup vs baseline: 1.2413x; 1.2413x over previous
"""Trainium2 Bass kernel for single-head self-attention over image tokens.

Reference computation (per batch element b of 4):
    xf   = x[b] viewed as [N=4096 tokens, C=256]          (x stored [C, H*W] = xf.T)
    qkv  = xf @ w_qkv.T                                   -> q, k, v each [N, 512]
    sim  = (q * 64**-0.5) @ k.T                           [N, N]
    attn = softmax(sim, axis=-1)
    out  = (attn @ v) @ w_out.T + b_out + xf              [N, C]

Sharding: 8 cores = 4 batches x 2 query-row halves (2048 rows each). Each core
computes k/v for its full batch but q/out only for its half. No collectives.
Each core's x is host-rotated so its query half is always columns 0:2048
(softmax over keys is permutation invariant, so key order doesn't matter).

Matmul operands use float32r: fp32 with the mantissa rounded to 11 bits
(round-half-even on the low 12 bits, same bit layout as fp32), which streams
1 PE column/cycle instead of 4 for plain fp32. x and the weights are
pre-rounded on the host and DMAed straight into float32r tiles; on-chip
intermediates (qT/kT/v/pT) get rounded by the PSUM->SBUF copy or activation
that produces them.

On-chip layout keeps everything in the "transposed activation" orientation so
no PE transposes are needed:
    qT [512, 2048] and kT [512, N] come straight out of the QKV projection
    (x's HBM layout [C, N] is already the rhs/lhsT the PE wants);
    v [N, 512] comes from the same projection with x slices as the stationary
    operand. simT [j, i] = kT.T @ qT, pT = exp(0.125*simT), then
    outT [d, i] += v_j.T @ pT accumulates in PSUM per 1024-column j-superblock
    and the softmax denominator l[1, i] += ones.T @ (pT pairs summed on
    GpSimd). Normalization is folded in at the end of the last superblock,
    per query slice: recip(l) via a fast Newton iteration on the DVE after a
    K=1 rank-1 broadcast matmul, multiplied into the final projection output.
"""

import hashlib
import os
import shutil

import numpy as np

import concourse.bacc as bacc
import concourse.tile as tile
import concourse.mybir as mybir
from concourse.bass_utils import run_bass_kernel_spmd


def _install_neff_cache():
    """Disk-cache walrus NEFF compiles keyed on the BIR content hash.

    The axon PJRT path recompiles the NEFF in every fresh process (~minutes);
    the build here is deterministic, so identical BIR -> identical NEFF.
    """
    try:
        import concourse.bass2jax as bass2jax
        orig = bass2jax.compile_bir_kernel
        if getattr(orig, "_neff_cache_wrapped", False):
            return
        cache_dir = os.path.expanduser("~/.neuron-compile-cache/bass-neff")

        def cached(bir_json, tmpdir, neff_name="file.neff"):
            try:
                key = hashlib.sha256(
                    bir_json if isinstance(bir_json, bytes)
                    else bir_json.encode()).hexdigest()
                hit = os.path.join(cache_dir, key + ".neff")
                dst = os.path.join(tmpdir, neff_name)
                if os.path.exists(hit):
                    shutil.copyfile(hit, dst)
                    return dst
                neff = orig(bir_json, tmpdir, neff_name=neff_name)
                os.makedirs(cache_dir, exist_ok=True)
                tmp = hit + ".tmp%d" % os.getpid()
                shutil.copyfile(neff, tmp)
                os.replace(tmp, hit)
                return neff
            except Exception:
                return orig(bir_json, tmpdir, neff_name=neff_name)

        cached._neff_cache_wrapped = True
        bass2jax.compile_bir_kernel = cached
    except Exception:
        pass


_install_neff_cache()

F32 = mybir.dt.float32
F32R = mybir.dt.float32r
F8E4 = mybir.dt.float8e4
F8E5 = mybir.dt.float8e5
DR = mybir.MatmulPerfMode.DoubleRow
Exp = mybir.ActivationFunctionType.Exp
SHIFT = 7.0  # exp(scale*sim - SHIFT): keeps pT <= e^10.1 < e5m2 max 57344;
             # cancels in out = po/l so no renormalization needed

B = 4
C = 256          # model dim (2 chunks of 128)
N = 4096         # tokens per batch (64*64)
HALF = N // 2    # query rows per core
INNER = 512      # qkv inner dim (4 chunks of 128)
SCALE = 0.125    # 64 ** -0.5

NCORES = 8
NJB = 4          # j superblocks per batch
JBW = N // NJB   # 1024 key columns per superblock
NSL = 4          # i slices per core
SW = HALF // NSL # 512 query columns per slice


def build_nc(n=N, njb=NJB, nsl=NSL):
    half = n // 2
    jbw = n // njb
    assert half % SW == 0 and jbw % SW == 0 and jbw % 256 == 0
    nc = bacc.Bacc(None)
    x_r = nc.declare_dram_parameter("x_r", [C, n], F32R, isOutput=False)
    xq_f = nc.declare_dram_parameter("xq_f", [C, half], F32, isOutput=False)
    wqkvT = nc.declare_dram_parameter("wqkvT", [C, 3 * INNER], F32R, isOutput=False)
    woutT = nc.declare_dram_parameter("woutT", [INNER, C], F32R, isOutput=False)
    bout = nc.declare_dram_parameter("bout", [2, 128, 1], F32, isOutput=False)
    out = nc.declare_dram_parameter("out", [C, half], F32, isOutput=True)

    mm = nc.tensor.matmul

    with tile.TileContext(nc) as tc:
        with tc.tile_pool(name="const", bufs=1) as const, \
             tc.tile_pool(name="stream", bufs=1) as stream, \
             tc.tile_pool(name="work", bufs=2) as work, \
             tc.tile_pool(name="pp", bufs=1, space="PSUM") as pp:

            # ---- resident weights: direct f32r DMA (host pre-rounded) ----
            wq = []
            for cc in range(2):
                t = const.tile([128, 3 * INNER], F32R, tag=f"wq{cc}", name=f"wq{cc}")
                nc.sync.dma_start(t, wqkvT[cc * 128:(cc + 1) * 128, :])
                wq.append(t)

            def xchunk(cc, col, width):
                """x chunk [128, width] in f32r, shares slots with xjb tiles."""
                t = stream.tile([128, width], F32R, tag=f"xjb{cc}", bufs=2,
                                name=f"xjb{cc}", padded_shape=[128, jbw])
                nc.sync.dma_start(t, x_r[cc * 128:(cc + 1) * 128, col:col + width])
                return t

            qT = [const.tile([128, half], F32R, tag=f"qt{d}", name=f"qt{d}")
                  for d in range(4)]
            ot = [const.tile([128, half], F32, tag=f"ot{d}", name=f"ot{d}")
                  for d in range(4)]
            l_sb = const.tile([1, half], F32, tag="l_sb", name="l_sb")

            ones_col_f = const.tile([128, 1], F32, tag="ones_col_f", name="ones_col_f")
            nc.vector.memset(ones_col_f, 1.0)
            ones_col = const.tile([128, 1], F32R, tag="ones_col", name="ones_col")
            nc.vector.tensor_copy(ones_col, ones_col_f)
            ones_row_f = const.tile([1, 128], F32, tag="ones_row_f", name="ones_row_f")
            nc.vector.memset(ones_row_f, 1.0)
            ones_row = const.tile([1, 128], F32R, tag="ones_row", name="ones_row")
            nc.vector.tensor_copy(ones_row, ones_row_f)
            nshift = const.tile([128, 1], F32, tag="nshift", name="nshift")
            nc.vector.memset(nshift, -SHIFT)

            # ---- qT production from x columns 0:half ----
            wo = []
            xqt = []
            bt = []
            qcw = min(1024, half)
            for qch in range(half // qcw):
                xch = [xchunk(cc, qch * qcw, qcw) for cc in range(2)]
                for d in range(4):
                    for nb in range(qcw // SW):
                        ns = qch * (qcw // SW) + nb
                        ps = pp.tile([128, SW], F32, tag="sim", bufs=3, name="ps_q")
                        for cc in range(2):
                            mm(ps, wq[cc][:, d * 128:(d + 1) * 128],
                               xch[cc][:, nb * SW:(nb + 1) * SW],
                               start=(cc == 0), stop=(cc == 1))
                        nc.scalar.copy(qT[d][:, ns * SW:(ns + 1) * SW], ps)
            # final-phase constants, off the startup critical path
            # (vector-queue DMAs so the sync queue stays free for x chunks)
            for d in range(4):
                t = const.tile([128, C], F32R, tag=f"wo{d}", name=f"wo{d}")
                nc.scalar.dma_start(t, woutT[d * 128:(d + 1) * 128, :])
                wo.append(t)
            for cc in range(2):
                t = const.tile([128, half], F32, tag=f"xq{cc}", name=f"xq{cc}")
                nc.scalar.dma_start(t, xq_f[cc * 128:(cc + 1) * 128, :])
                xqt.append(t)
            for cc in range(2):
                t = const.tile([128, 1], F32, tag=f"b{cc}", name=f"b{cc}")
                nc.scalar.dma_start(t, bout[cc])
                bt.append(t)

            # residual-with-bias: xqt <- xqt + b
            for cc in range(2):
                nc.vector.tensor_scalar_add(xqt[cc], xqt[cc], bt[cc])

            # ---- attention over j superblocks ----
            deferred = []  # denominator work deferred into later PE streams
            for jb in range(njb):
                xjb = [xchunk(cc, jb * jbw, jbw) for cc in range(2)]
                # kT for this superblock: [512, jbw]
                kt = [stream.tile([128, jbw], F32R, tag=f"kt{d}", bufs=1,
                                  name=f"kt{d}") for d in range(4)]
                for d in range(4):
                    for nb in range(jbw // SW):
                        ps = pp.tile([128, SW], F32, tag="sim", bufs=3, name="ps_k")
                        for cc in range(2):
                            mm(ps, wq[cc][:, INNER + d * 128:INNER + (d + 1) * 128],
                               xjb[cc][:, nb * SW:(nb + 1) * SW],
                               start=(cc == 0), stop=(cc == 1))
                        nc.scalar.copy(kt[d][:, nb * SW:(nb + 1) * SW], ps)
                # v for this superblock: [jbw, 512] (token rows on partitions),
                # stored fp8e4 in token-pair planes for DoubleRow po matmuls
                vtp = []
                for t2 in range(jbw // 256):
                    t = stream.tile([128, 2, INNER], F8E4, tag=f"vt{t2}", bufs=1,
                                    name=f"vt{t2}")
                    vtp.append(t)
                for nj in range(jbw // 128):
                    ps = pp.tile([128, INNER], F32, tag="sim", bufs=3, name="ps_v")
                    for cc in range(2):
                        mm(ps, xjb[cc][:, nj * 128:(nj + 1) * 128],
                           wq[cc][:, 2 * INNER:3 * INNER],
                           start=(cc == 0), stop=(cc == 1))
                    nc.scalar.copy(vtp[nj // 2][:, nj % 2, :], ps)

                for fn in deferred:
                    fn()
                deferred.clear()

                nj8 = jbw // 128
                npair = nj8 // 2
                for s in range(nsl):
                    sl = slice(s * SW, (s + 1) * SW)
                    po = [pp.tile([128, SW], F32, tag=f"po{d}", bufs=1,
                                  name=f"po{d}") for d in range(4)]
                    pl = pp.tile([1, SW], F32, tag="aux", bufs=1, name="pl")
                    ptp = []

                    sums = []  # binary tree of pT pair-sums (DVE)

                    def tree_add(t):
                        sums.append([t, 0])
                        while len(sums) >= 2 and sums[-1][1] == sums[-2][1]:
                            a, lv = sums.pop()
                            b, _ = sums.pop()
                            t2 = work.tile([128, SW], F32R, tag="pt2", bufs=4,
                                           name="pt2")
                            nc.vector.tensor_add(t2, b, a)
                            sums.append([t2, lv + 1])

                    last_jb = jb == njb - 1

                    def l_update(jb=jb, sl=sl, pl=pl):
                        if jb == 0:
                            nc.vector.tensor_copy(l_sb[:, sl], pl)
                        else:
                            nc.vector.tensor_add(l_sb[:, sl], l_sb[:, sl], pl)

                    def drain_pair(p):
                        # outT + denominator work for token-pair p (emitted a
                        # pair late so the PE never waits on the exp). po is a
                        # DoubleRow fp8 matmul: 256 tokens contracted per mm.
                        for d in range(4):
                            mm(po[d], vtp[p][:, :, d * 128:(d + 1) * 128],
                               ptp[p][:, :, :],
                               start=(p == 0), stop=(p == npair - 1),
                               perf_mode=DR)
                        pt2 = work.tile([128, SW], F32R, tag="pt2",
                                        bufs=4, name="pt2")
                        nc.vector.tensor_add(pt2, ptp[p][:, 0, :],
                                             ptp[p][:, 1, :])
                        if last_jb:
                            # inline: the finalize below needs l with no
                            # deferral room
                            mm(pl, ones_col, pt2,
                               start=(p == 0), stop=(p == npair - 1))
                        else:
                            # tree-sum the pair sums on the DVE; the single
                            # ones-matmul + l update are deferred into a later
                            # PE stream so the PE never waits on the adder tree
                            tree_add(pt2)
                            if p == npair - 1:
                                assert len(sums) == 1
                                pt8 = sums[0][0]

                                def flush(pl=pl, pt8=pt8, upd=l_update):
                                    mm(pl, ones_col, pt8, start=True, stop=True)
                                    upd()
                                deferred.append(flush)

                    for j8 in range(nj8):
                        ps = pp.tile([128, SW], F32, tag="sim", bufs=3, name="ps_s")
                        for d in range(4):
                            mm(ps, kt[d][:, j8 * 128:(j8 + 1) * 128], qT[d][:, sl],
                               start=(d == 0), stop=(d == 3))
                        p, parity = divmod(j8, 2)
                        if parity == 0:
                            t = work.tile([128, 2, SW], F8E5, tag="pt", bufs=4,
                                          name="pt")
                            ptp.append(t)
                        nc.scalar.activation(ptp[p][:, parity, :], ps, Exp,
                                             scale=SCALE, bias=nshift)
                        if parity == 1 and p > 0:
                            drain_pair(p - 1)
                        if j8 == 2:
                            for fn in deferred:
                                fn()
                            deferred.clear()
                    drain_pair(npair - 1)
                    if last_jb:
                        l_update()
                    for d in range(4):
                        if jb == 0:
                            nc.vector.tensor_copy(ot[d][:, sl], po[d])
                        else:
                            nc.vector.tensor_add(ot[d][:, sl], ot[d][:, sl], po[d])

                    if jb == njb - 1:
                        # ---- finalize slice s: normalize + project + out ----
                        l_rs = work.tile([1, SW], F32R, tag="l_rs", bufs=2,
                                         name="l_rs")
                        nc.scalar.copy(l_rs, l_sb[:, sl])
                        pb = pp.tile([128, SW], F32, tag="sim", bufs=3, name="pb")
                        mm(pb, ones_row, l_rs, start=True, stop=True)
                        bc = work.tile([128, SW], F32, tag="bc", bufs=2, name="bc")
                        rsc = work.tile([128, SW], F32, tag="rsc", bufs=2,
                                        name="rsc")
                        nc.vector.reciprocal_approx_accurate(bc, pb, rsc)
                        otr = [work.tile([128, SW], F32R, tag=f"otr{d}", bufs=1,
                                         name=f"otr{d}") for d in range(4)]
                        for d in range(4):
                            nc.scalar.copy(otr[d], ot[d][:, sl])
                        for cc in range(2):
                            pf = pp.tile([128, SW], F32, tag="sim", bufs=3,
                                         name="pf")
                            for d in range(4):
                                mm(pf, wo[d][:, cc * 128:(cc + 1) * 128], otr[d],
                                   start=(d == 0), stop=(d == 3))
                            fo = work.tile([128, SW], F32, tag="fo", bufs=2,
                                           name="fo")
                            nc.vector.tensor_mul(fo, pf, bc)
                            nc.vector.tensor_add(fo, fo, xqt[cc][:, sl])
                            nc.sync.dma_start(out[cc * 128:(cc + 1) * 128, sl], fo)

    nc.finalize()
    return nc


_NC_CACHE = None


def _get_nc():
    global _NC_CACHE
    if _NC_CACHE is None:
        _NC_CACHE = build_nc()
    return _NC_CACHE


def _round_f32r(a):
    """fp32 -> float32r rounding (round-half-even on the low 12 mantissa
    bits), matching the hardware's fp32_to_fp32r conversion."""
    bits = np.ascontiguousarray(a, dtype=np.float32).view(np.uint32)
    rem = bits & np.uint32(0xFFF)
    base = bits & np.uint32(0xFFFFF000)
    up = (rem > 0x800) | ((rem == 0x800) & (((bits >> np.uint32(12)) & np.uint32(1)) == 1))
    return (base + np.where(up, np.uint32(0x1000), np.uint32(0))).view(np.float32)


def prepare_in_maps(x, w_qkv, w_out, b_out):
    x = np.asarray(x, dtype=np.float32)
    w_qkv = np.asarray(w_qkv, dtype=np.float32)
    w_out = np.asarray(w_out, dtype=np.float32)
    b_out = np.asarray(b_out, dtype=np.float32)

    xr = x.reshape(B, C, N)
    wqkvT = _round_f32r(np.ascontiguousarray(w_qkv.T))   # [C, 1536]
    woutT = _round_f32r(np.ascontiguousarray(w_out.T))   # [512, C]
    bout = np.ascontiguousarray(b_out.reshape(2, 128, 1))

    in_maps = []
    for c in range(NCORES):
        b, h = divmod(c, 2)
        if h == 0:
            x_rot = xr[b]
        else:  # rotate so this core's query half sits in columns 0:HALF
            x_rot = np.concatenate([xr[b][:, HALF:], xr[b][:, :HALF]], axis=1)
        in_maps.append({
            "x_r": _round_f32r(x_rot),
            "xq_f": np.ascontiguousarray(x_rot[:, :HALF]),
            "wqkvT": wqkvT,
            "woutT": woutT,
            "bout": bout,
        })
    return in_maps


def postprocess(results):
    outs = [results[c]["out"] for c in range(NCORES)]
    full = np.stack([np.concatenate([outs[2 * b], outs[2 * b + 1]], axis=1)
                     for b in range(B)])               # [B, C, N]
    return full.reshape(B, C, 64, 64).astype(np.float32)


def kernel(x, w_qkv, w_out, b_out):
    in_maps = prepare_in_maps(x, w_qkv, w_out, b_out)
    res = run_bass_kernel_spmd(_get_nc(), in_maps, core_ids=list(range(NCORES)))
    return postprocess(res.results)



# revision 15
# speedup vs baseline: 1.2902x; 1.0394x over previous
"""Trainium2 Bass kernel for single-head self-attention over image tokens.

Reference computation (per batch element b of 4):
    xf   = x[b] viewed as [N=4096 tokens, C=256]          (x stored [C, H*W] = xf.T)
    qkv  = xf @ w_qkv.T                                   -> q, k, v each [N, 512]
    sim  = (q * 64**-0.5) @ k.T                           [N, N]
    attn = softmax(sim, axis=-1)
    out  = (attn @ v) @ w_out.T + b_out + xf              [N, C]

Sharding: 8 cores = 4 batches x 2 query-row halves (2048 rows each). Each core
computes k/v for its full batch but q/out only for its half. No collectives.
Each core's x is host-rotated so its query half is always columns 0:2048
(softmax over keys is permutation invariant, so key order doesn't matter).

Matmul operands use float32r: fp32 with the mantissa rounded to 11 bits
(round-half-even on the low 12 bits, same bit layout as fp32), which streams
1 PE column/cycle instead of 4 for plain fp32. x and the weights are
pre-rounded on the host and DMAed straight into float32r tiles; on-chip
intermediates (qT/kT/v/pT) get rounded by the PSUM->SBUF copy or activation
that produces them.

On-chip layout keeps everything in the "transposed activation" orientation so
no PE transposes are needed:
    qT [512, 2048] and kT [512, N] come straight out of the QKV projection
    (x's HBM layout [C, N] is already the rhs/lhsT the PE wants);
    v [N, 512] comes from the same projection with x slices as the stationary
    operand. simT [j, i] = kT.T @ qT, pT = exp(0.125*simT), then
    outT [d, i] += v_j.T @ pT accumulates in PSUM per 1024-column j-superblock
    and the softmax denominator l[1, i] += ones.T @ (pT pairs summed on
    GpSimd). Normalization is folded in at the end of the last superblock,
    per query slice: recip(l) via a fast Newton iteration on the DVE after a
    K=1 rank-1 broadcast matmul, multiplied into the final projection output.
"""

import hashlib
import os
import shutil

import numpy as np

import concourse.bacc as bacc
import concourse.tile as tile
import concourse.mybir as mybir
from concourse.bass_utils import run_bass_kernel_spmd


def _install_neff_cache():
    """Disk-cache walrus NEFF compiles keyed on the BIR content hash.

    The axon PJRT path recompiles the NEFF in every fresh process (~minutes);
    the build here is deterministic, so identical BIR -> identical NEFF.
    """
    try:
        import concourse.bass2jax as bass2jax
        orig = bass2jax.compile_bir_kernel
        if getattr(orig, "_neff_cache_wrapped", False):
            return
        cache_dir = os.path.expanduser("~/.neuron-compile-cache/bass-neff")

        def cached(bir_json, tmpdir, neff_name="file.neff"):
            try:
                key = hashlib.sha256(
                    bir_json if isinstance(bir_json, bytes)
                    else bir_json.encode()).hexdigest()
                hit = os.path.join(cache_dir, key + ".neff")
                dst = os.path.join(tmpdir, neff_name)
                if os.path.exists(hit):
                    shutil.copyfile(hit, dst)
                    return dst
                neff = orig(bir_json, tmpdir, neff_name=neff_name)
                os.makedirs(cache_dir, exist_ok=True)
                tmp = hit + ".tmp%d" % os.getpid()
                shutil.copyfile(neff, tmp)
                os.replace(tmp, hit)
                return neff
            except Exception:
                return orig(bir_json, tmpdir, neff_name=neff_name)

        cached._neff_cache_wrapped = True
        bass2jax.compile_bir_kernel = cached
    except Exception:
        pass


_install_neff_cache()

F32 = mybir.dt.float32
F32R = mybir.dt.float32r
F8E4 = mybir.dt.float8e4
F8E5 = mybir.dt.float8e5
DR = mybir.MatmulPerfMode.DoubleRow
Exp = mybir.ActivationFunctionType.Exp
SHIFT = 7.0  # exp(scale*sim - SHIFT): keeps pT <= e^10.1 < e5m2 max 57344;
             # cancels in out = po/l so no renormalization needed

B = 4
C = 256          # model dim (2 chunks of 128)
N = 4096         # tokens per batch (64*64)
HALF = N // 2    # query rows per core
INNER = 512      # qkv inner dim (4 chunks of 128)
SCALE = 0.125    # 64 ** -0.5

NCORES = 8
NJB = 4          # j superblocks per batch
JBW = N // NJB   # 1024 key columns per superblock
NSL = 4          # i slices per core
SW = HALF // NSL # 512 query columns per slice


def build_nc(n=N, njb=NJB, nsl=NSL):
    half = n // 2
    jbw = n // njb
    assert half % SW == 0 and jbw % SW == 0 and jbw % 256 == 0
    nc = bacc.Bacc(None)
    x_r = nc.declare_dram_parameter("x_r", [C, n], F32R, isOutput=False)
    xq_f = nc.declare_dram_parameter("xq_f", [C, half], F32, isOutput=False)
    wqkvT = nc.declare_dram_parameter("wqkvT", [C, 3 * INNER], F32R, isOutput=False)
    woutT = nc.declare_dram_parameter("woutT", [INNER, C], F32R, isOutput=False)
    bout = nc.declare_dram_parameter("bout", [2, 128, 1], F32, isOutput=False)
    out = nc.declare_dram_parameter("out", [C, half], F32, isOutput=True)

    mm = nc.tensor.matmul

    with tile.TileContext(nc) as tc:
        with tc.tile_pool(name="const", bufs=1) as const, \
             tc.tile_pool(name="stream", bufs=1) as stream, \
             tc.tile_pool(name="work", bufs=2) as work, \
             tc.tile_pool(name="pp", bufs=1, space="PSUM") as pp:

            # ---- resident weights: direct f32r DMA (host pre-rounded) ----
            # split per q/k/v so the first qT matmul only waits on the q part
            wqq, wqk, wqv = [], [], []
            for part, lst in ((0, wqq), (1, wqk), (2, wqv)):
                for cc in range(2):
                    t = const.tile([128, INNER], F32R, tag=f"wq{part}{cc}",
                                   name=f"wq{part}{cc}")
                    nc.sync.dma_start(
                        t, wqkvT[cc * 128:(cc + 1) * 128,
                                 part * INNER:(part + 1) * INNER])
                    lst.append(t)

            def xchunk(cc, col, width):
                """x chunk [128, width] in f32r, shares slots with xjb tiles.
                GpSimd-queue DMA: runs in parallel with the weight stream on
                the sync queue."""
                t = stream.tile([128, width], F32R, tag=f"xjb{cc}", bufs=2,
                                name=f"xjb{cc}", padded_shape=[128, jbw])
                nc.gpsimd.dma_start(t, x_r[cc * 128:(cc + 1) * 128, col:col + width])
                return t

            qT = [const.tile([128, half], F32R, tag=f"qt{d}", name=f"qt{d}")
                  for d in range(4)]
            ot = [const.tile([128, half], F32, tag=f"ot{d}", name=f"ot{d}")
                  for d in range(4)]
            l_sb = const.tile([1, half], F32, tag="l_sb", name="l_sb")

            ones_col_f = const.tile([128, 1], F32, tag="ones_col_f", name="ones_col_f")
            nc.vector.memset(ones_col_f, 1.0)
            ones_col = const.tile([128, 1], F32R, tag="ones_col", name="ones_col")
            nc.vector.tensor_copy(ones_col, ones_col_f)
            ones_row_f = const.tile([1, 128], F32, tag="ones_row_f", name="ones_row_f")
            nc.vector.memset(ones_row_f, 1.0)
            ones_row = const.tile([1, 128], F32R, tag="ones_row", name="ones_row")
            nc.vector.tensor_copy(ones_row, ones_row_f)
            nshift = const.tile([128, 1], F32, tag="nshift", name="nshift")
            nc.vector.memset(nshift, -SHIFT)
            ones_sq_f = const.tile([128, 128], F32, tag="ones_sq_f",
                                   name="ones_sq_f")
            nc.vector.memset(ones_sq_f, 1.0)
            ones_sq = const.tile([128, 128], F32R, tag="ones_sq", name="ones_sq")
            nc.vector.tensor_copy(ones_sq, ones_sq_f)

            # ---- qT production from x columns 0:half ----
            wo = []
            xqt = []
            bt = []
            qcw = SW  # small first blocks: compute starts after ~0.75MB of DMA
            for qch in range(half // qcw):
                xch = [xchunk(cc, qch * qcw, qcw) for cc in range(2)]
                for d in range(4):
                    ps = pp.tile([128, SW], F32, tag="sim", bufs=3, name="ps_q")
                    for cc in range(2):
                        mm(ps, wqq[cc][:, d * 128:(d + 1) * 128],
                           xch[cc][:, 0:SW],
                           start=(cc == 0), stop=(cc == 1))
                    nc.scalar.copy(qT[d][:, qch * SW:(qch + 1) * SW], ps)
            # final-phase constants, off the startup critical path
            # (vector-queue DMAs so the sync queue stays free for x chunks)
            for d in range(4):
                t = const.tile([128, C], F32R, tag=f"wo{d}", name=f"wo{d}")
                nc.scalar.dma_start(t, woutT[d * 128:(d + 1) * 128, :])
                wo.append(t)
            for cc in range(2):
                t = const.tile([128, half], F32, tag=f"xq{cc}", name=f"xq{cc}")
                nc.scalar.dma_start(t, xq_f[cc * 128:(cc + 1) * 128, :])
                xqt.append(t)
            for cc in range(2):
                t = const.tile([128, 1], F32, tag=f"b{cc}", name=f"b{cc}")
                nc.scalar.dma_start(t, bout[cc])
                bt.append(t)

            # residual-with-bias: xqt <- xqt + b
            for cc in range(2):
                nc.vector.tensor_scalar_add(xqt[cc], xqt[cc], bt[cc])

            # ---- attention over j superblocks ----
            deferred = []  # denominator work deferred into later PE streams
            for jb in range(njb):
                xjb = [xchunk(cc, jb * jbw, jbw) for cc in range(2)]
                # kT for this superblock: [512, jbw]
                kt = [stream.tile([128, jbw], F32R, tag=f"kt{d}", bufs=1,
                                  name=f"kt{d}") for d in range(4)]
                for d in range(4):
                    for nb in range(jbw // SW):
                        ps = pp.tile([128, SW], F32, tag="sim", bufs=3, name="ps_k")
                        for cc in range(2):
                            mm(ps, wqk[cc][:, d * 128:(d + 1) * 128],
                               xjb[cc][:, nb * SW:(nb + 1) * SW],
                               start=(cc == 0), stop=(cc == 1))
                        nc.scalar.copy(kt[d][:, nb * SW:(nb + 1) * SW], ps)
                # v for this superblock: [jbw, 512] (token rows on partitions),
                # stored fp8e4 in token-pair planes for DoubleRow po matmuls
                vtp = []
                for t2 in range(jbw // 256):
                    t = stream.tile([128, 2, INNER], F8E4, tag=f"vt{t2}", bufs=1,
                                    name=f"vt{t2}")
                    vtp.append(t)
                for nj in range(jbw // 128):
                    ps = pp.tile([128, INNER], F32, tag="sim", bufs=3, name="ps_v")
                    for cc in range(2):
                        mm(ps, xjb[cc][:, nj * 128:(nj + 1) * 128],
                           wqv[cc][:, :],
                           start=(cc == 0), stop=(cc == 1))
                    nc.scalar.copy(vtp[nj // 2][:, nj % 2, :], ps)

                for fn in deferred:
                    fn()
                deferred.clear()

                nj8 = jbw // 128
                npair = nj8 // 2
                for s in range(nsl):
                    sl = slice(s * SW, (s + 1) * SW)
                    last_jb = jb == njb - 1
                    po = [pp.tile([128, SW], F32, tag=f"po{d}", bufs=1,
                                  name=f"po{d}") for d in range(4)]
                    if not last_jb:
                        pl = pp.tile([1, SW], F32, tag="aux", bufs=1, name="pl")
                    else:
                        pl = None
                        # l through jb 0..2 in f32r, off the critical path
                        l_rs = work.tile([1, SW], F32R, tag="l_rs", bufs=2,
                                         name="l_rs")
                        nc.scalar.copy(l_rs, l_sb[:, sl])
                    ptp = []

                    sums = []  # binary tree of pT pair-sums (DVE)

                    def tree_add(t):
                        sums.append([t, 0])
                        while len(sums) >= 2 and sums[-1][1] == sums[-2][1]:
                            a, lv = sums.pop()
                            b, _ = sums.pop()
                            t2 = work.tile([128, SW], F32R, tag="pt2", bufs=4,
                                           name="pt2")
                            nc.vector.tensor_add(t2, b, a)
                            sums.append([t2, lv + 1])

                    def l_update(jb=jb, sl=sl, pl=pl):
                        if jb == 0:
                            nc.vector.tensor_copy(l_sb[:, sl], pl)
                        else:
                            nc.vector.tensor_add(l_sb[:, sl], l_sb[:, sl], pl)

                    def drain_pair(p):
                        # outT + denominator work for token-pair p (emitted a
                        # pair late so the PE never waits on the exp). po is a
                        # DoubleRow fp8 matmul: 256 tokens contracted per mm.
                        for d in range(4):
                            mm(po[d], vtp[p][:, :, d * 128:(d + 1) * 128],
                               ptp[p][:, :, :],
                               start=(p == 0), stop=(p == npair - 1),
                               perf_mode=DR)
                        # tree-sum the pair sums on the DVE; the ones-matmul +
                        # l update (non-last jb) or the finalize (last jb) are
                        # deferred into a later PE stream so the PE never
                        # waits on the adder tree
                        pt2 = work.tile([128, SW], F32R, tag="pt2",
                                        bufs=4, name="pt2")
                        nc.vector.tensor_add(pt2, ptp[p][:, 0, :],
                                             ptp[p][:, 1, :])
                        tree_add(pt2)
                        if p == npair - 1 and not last_jb:
                            assert len(sums) == 1
                            pt8 = sums[0][0]

                            def flush(pl=pl, pt8=pt8, upd=l_update):
                                mm(pl, ones_col, pt8, start=True, stop=True)
                                upd()
                            deferred.append(flush)

                    for j8 in range(nj8):
                        ps = pp.tile([128, SW], F32, tag="sim", bufs=3, name="ps_s")
                        for d in range(4):
                            mm(ps, kt[d][:, j8 * 128:(j8 + 1) * 128], qT[d][:, sl],
                               start=(d == 0), stop=(d == 3))
                        p, parity = divmod(j8, 2)
                        if parity == 0:
                            t = work.tile([128, 2, SW], F8E5, tag="pt", bufs=4,
                                          name="pt")
                            ptp.append(t)
                        nc.scalar.activation(ptp[p][:, parity, :], ps, Exp,
                                             scale=SCALE, bias=nshift)
                        if parity == 1 and p > 0:
                            drain_pair(p - 1)
                        if j8 == 2:
                            for fn in deferred:
                                fn()
                            deferred.clear()
                    drain_pair(npair - 1)
                    if not last_jb:
                        for d in range(4):
                            if jb == 0:
                                nc.vector.tensor_copy(ot[d][:, sl], po[d])
                            else:
                                nc.vector.tensor_add(ot[d][:, sl], ot[d][:, sl],
                                                     po[d])
                    else:
                        # ---- finalize slice s: normalize + project + out ----
                        assert len(sums) == 1
                        pt8 = sums[0][0]
                        # otr = ot (jb 0..2) + po (jb 3), fused accumulate +
                        # f32r convert, emitted now so the DVE adds overlap the
                        # next slice's sim stream
                        otr = [work.tile([128, SW], F32R, tag=f"otr{d}", bufs=1,
                                         name=f"otr{d}") for d in range(4)]
                        for d in range(4):
                            nc.vector.tensor_add(otr[d], ot[d][:, sl], po[d])

                        def finalize(sl=sl, pt8=pt8, l_rs=l_rs, otr=otr):
                            # total l broadcast to 128 partitions in one psum
                            # accumulation: colsum(pt8) via all-ones stationary
                            # + broadcast of l_sb (jb 0..2) via ones_row
                            pb = pp.tile([128, SW], F32, tag="aux", bufs=1,
                                         name="pb")
                            mm(pb, ones_sq, pt8, start=True, stop=False)
                            mm(pb, ones_row, l_rs, start=False, stop=True)
                            bc = work.tile([128, SW], F32, tag="bc", bufs=2,
                                           name="bc")
                            rsc = work.tile([128, SW], F32, tag="rsc", bufs=2,
                                            name="rsc")
                            nc.vector.reciprocal_approx_accurate(bc, pb, rsc)
                            for cc in range(2):
                                pf = pp.tile([128, SW], F32, tag="sim", bufs=3,
                                             name="pf")
                                for d in range(4):
                                    mm(pf, wo[d][:, cc * 128:(cc + 1) * 128],
                                       otr[d], start=(d == 0), stop=(d == 3))
                                fo = work.tile([128, SW], F32, tag="fo", bufs=2,
                                               name="fo")
                                nc.vector.tensor_mul(fo, pf, bc)
                                nc.vector.tensor_add(fo, fo, xqt[cc][:, sl])
                                nc.sync.dma_start(out[cc * 128:(cc + 1) * 128, sl],
                                                  fo)
                        if s == nsl - 1:
                            finalize()
                        else:
                            deferred.append(finalize)

    nc.finalize()
    return nc


_NC_CACHE = None


def _get_nc():
    global _NC_CACHE
    if _NC_CACHE is None:
        _NC_CACHE = build_nc()
    return _NC_CACHE


def _round_f32r(a):
    """fp32 -> float32r rounding (round-half-even on the low 12 mantissa
    bits), matching the hardware's fp32_to_fp32r conversion."""
    bits = np.ascontiguousarray(a, dtype=np.float32).view(np.uint32)
    rem = bits & np.uint32(0xFFF)
    base = bits & np.uint32(0xFFFFF000)
    up = (rem > 0x800) | ((rem == 0x800) & (((bits >> np.uint32(12)) & np.uint32(1)) == 1))
    return (base + np.where(up, np.uint32(0x1000), np.uint32(0))).view(np.float32)


def prepare_in_maps(x, w_qkv, w_out, b_out):
    x = np.asarray(x, dtype=np.float32)
    w_qkv = np.asarray(w_qkv, dtype=np.float32)
    w_out = np.asarray(w_out, dtype=np.float32)
    b_out = np.asarray(b_out, dtype=np.float32)

    xr = x.reshape(B, C, N)
    wqkvT = _round_f32r(np.ascontiguousarray(w_qkv.T))   # [C, 1536]
    woutT = _round_f32r(np.ascontiguousarray(w_out.T))   # [512, C]
    bout = np.ascontiguousarray(b_out.reshape(2, 128, 1))

    in_maps = []
    for c in range(NCORES):
        b, h = divmod(c, 2)
        if h == 0:
            x_rot = xr[b]
        else:  # rotate so this core's query half sits in columns 0:HALF
            x_rot = np.concatenate([xr[b][:, HALF:], xr[b][:, :HALF]], axis=1)
        in_maps.append({
            "x_r": _round_f32r(x_rot),
            "xq_f": np.ascontiguousarray(x_rot[:, :HALF]),
            "wqkvT": wqkvT,
            "woutT": woutT,
            "bout": bout,
        })
    return in_maps


def postprocess(results):
    outs = [results[c]["out"] for c in range(NCORES)]
    full = np.stack([np.concatenate([outs[2 * b], outs[2 * b + 1]], axis=1)
                     for b in range(B)])               # [B, C, N]
    return full.reshape(B, C, 64, 64).astype(np.float32)


def kernel(x, w_qkv, w_out, b_out):
    in_maps = prepare_in_maps(x, w_qkv, w_out, b_out)
    res = run_bass_kernel_spmd(_get_nc(), in_maps, core_ids=list(range(NCORES)))
    return postprocess(res.results)



# revision 18
# speedup vs baseline: 1.3143x; 1.0187x over previous
"""Trainium2 Bass kernel for single-head self-attention over image tokens.

Reference computation (per batch element b of 4):
    xf   = x[b] viewed as [N=4096 tokens, C=256]          (x stored [C, H*W] = xf.T)
    qkv  = xf @ w_qkv.T                                   -> q, k, v each [N, 512]
    sim  = (q * 64**-0.5) @ k.T                           [N, N]
    attn = softmax(sim, axis=-1)
    out  = (attn @ v) @ w_out.T + b_out + xf              [N, C]

Sharding: 8 cores = 4 batches x 2 query-row halves (2048 rows each). Each core
computes k/v for its full batch but q/out only for its half. No collectives.
Each core's x is host-rotated so its query half is always columns 0:2048
(softmax over keys is permutation invariant, so key order doesn't matter).

Matmul operands use float32r: fp32 with the mantissa rounded to 11 bits
(round-half-even on the low 12 bits, same bit layout as fp32), which streams
1 PE column/cycle instead of 4 for plain fp32. x and the weights are
pre-rounded on the host and DMAed straight into float32r tiles; on-chip
intermediates (qT/kT/v/pT) get rounded by the PSUM->SBUF copy or activation
that produces them.

On-chip layout keeps everything in the "transposed activation" orientation so
no PE transposes are needed:
    qT [512, 2048] and kT [512, N] come straight out of the QKV projection
    (x's HBM layout [C, N] is already the rhs/lhsT the PE wants);
    v [N, 512] comes from the same projection with x slices as the stationary
    operand. simT [j, i] = kT.T @ qT, pT = exp(0.125*simT), then
    outT [d, i] += v_j.T @ pT accumulates in PSUM per 1024-column j-superblock
    and the softmax denominator l[1, i] += ones.T @ (pT pairs summed on
    GpSimd). Normalization is folded in at the end of the last superblock,
    per query slice: recip(l) via a fast Newton iteration on the DVE after a
    K=1 rank-1 broadcast matmul, multiplied into the final projection output.
"""

import hashlib
import os
import shutil

import numpy as np

import concourse.bacc as bacc
import concourse.tile as tile
import concourse.mybir as mybir
from concourse.bass_utils import run_bass_kernel_spmd


def _install_neff_cache():
    """Disk-cache walrus NEFF compiles keyed on the BIR content hash.

    The axon PJRT path recompiles the NEFF in every fresh process (~minutes);
    the build here is deterministic, so identical BIR -> identical NEFF.
    """
    try:
        import concourse.bass2jax as bass2jax
        orig = bass2jax.compile_bir_kernel
        if getattr(orig, "_neff_cache_wrapped", False):
            return
        cache_dir = os.path.expanduser("~/.neuron-compile-cache/bass-neff")

        def cached(bir_json, tmpdir, neff_name="file.neff"):
            try:
                key = hashlib.sha256(
                    bir_json if isinstance(bir_json, bytes)
                    else bir_json.encode()).hexdigest()
                hit = os.path.join(cache_dir, key + ".neff")
                dst = os.path.join(tmpdir, neff_name)
                if os.path.exists(hit):
                    shutil.copyfile(hit, dst)
                    return dst
                neff = orig(bir_json, tmpdir, neff_name=neff_name)
                os.makedirs(cache_dir, exist_ok=True)
                tmp = hit + ".tmp%d" % os.getpid()
                shutil.copyfile(neff, tmp)
                os.replace(tmp, hit)
                return neff
            except Exception:
                return orig(bir_json, tmpdir, neff_name=neff_name)

        cached._neff_cache_wrapped = True
        bass2jax.compile_bir_kernel = cached
    except Exception:
        pass


_install_neff_cache()

F32 = mybir.dt.float32
F32R = mybir.dt.float32r
F8E4 = mybir.dt.float8e4
F8E5 = mybir.dt.float8e5
DR = mybir.MatmulPerfMode.DoubleRow
Exp = mybir.ActivationFunctionType.Exp
SHIFT = 7.0  # exp(scale*sim - SHIFT): keeps pT <= e^10.1 < e5m2 max 57344;
             # cancels in out = po/l so no renormalization needed

B = 4
C = 256          # model dim (2 chunks of 128)
N = 4096         # tokens per batch (64*64)
HALF = N // 2    # query rows per core
INNER = 512      # qkv inner dim (4 chunks of 128)
SCALE = 0.125    # 64 ** -0.5

NCORES = 8
NJB = 4          # j superblocks per batch
JBW = N // NJB   # 1024 key columns per superblock
NSL = 4          # i slices per core
SW = HALF // NSL # 512 query columns per slice


def build_nc(n=N, njb=NJB, nsl=NSL):
    half = n // 2
    jbw = n // njb
    assert half % SW == 0 and jbw % SW == 0 and jbw % 256 == 0
    nc = bacc.Bacc(None)
    x_r = nc.declare_dram_parameter("x_r", [C, n], F32R, isOutput=False)
    xq_f = nc.declare_dram_parameter("xq_f", [C, half], F32, isOutput=False)
    wqkvT = nc.declare_dram_parameter("wqkvT", [C, 3 * INNER], F32R, isOutput=False)
    woutT = nc.declare_dram_parameter("woutT", [INNER, C], F32R, isOutput=False)
    bout = nc.declare_dram_parameter("bout", [2, 128, 1], F32, isOutput=False)
    out = nc.declare_dram_parameter("out", [C, half], F32, isOutput=True)

    mm = nc.tensor.matmul

    with tile.TileContext(nc) as tc:
        with tc.tile_pool(name="const", bufs=1) as const, \
             tc.tile_pool(name="stream", bufs=1) as stream, \
             tc.tile_pool(name="work", bufs=2) as work, \
             tc.tile_pool(name="pp", bufs=1, space="PSUM") as pp:

            # ---- resident weights: direct f32r DMA (host pre-rounded) ----
            # split per q/k/v so the first qT matmul only waits on the q part
            wqq, wqk, wqv = [], [], []
            for part, lst in ((0, wqq), (1, wqk), (2, wqv)):
                for cc in range(2):
                    t = const.tile([128, INNER], F32R, tag=f"wq{part}{cc}",
                                   name=f"wq{part}{cc}")
                    nc.sync.dma_start(
                        t, wqkvT[cc * 128:(cc + 1) * 128,
                                 part * INNER:(part + 1) * INNER])
                    lst.append(t)

            def xchunk(cc, col, width):
                """x chunk [128, width] in f32r, shares slots with xjb tiles.
                GpSimd-queue DMA: runs in parallel with the weight stream on
                the sync queue."""
                t = stream.tile([128, width], F32R, tag=f"xjb{cc}", bufs=4,
                                name=f"xjb{cc}", padded_shape=[128, jbw])
                nc.gpsimd.dma_start(t, x_r[cc * 128:(cc + 1) * 128, col:col + width])
                return t

            qT = [const.tile([128, half], F32R, tag=f"qt{d}", name=f"qt{d}")
                  for d in range(4)]
            ot = [const.tile([128, half], F32, tag=f"ot{d}", name=f"ot{d}")
                  for d in range(4)]
            l_sb = const.tile([1, half], F32, tag="l_sb", name="l_sb")

            ones_col_f = const.tile([128, 1], F32, tag="ones_col_f", name="ones_col_f")
            nc.vector.memset(ones_col_f, 1.0)
            ones_col = const.tile([128, 1], F32R, tag="ones_col", name="ones_col")
            nc.vector.tensor_copy(ones_col, ones_col_f)
            ones_row_f = const.tile([1, 128], F32, tag="ones_row_f", name="ones_row_f")
            nc.vector.memset(ones_row_f, 1.0)
            ones_row = const.tile([1, 128], F32R, tag="ones_row", name="ones_row")
            nc.vector.tensor_copy(ones_row, ones_row_f)
            nshift = const.tile([128, 1], F32, tag="nshift", name="nshift")
            nc.vector.memset(nshift, -SHIFT)
            ones_sq_f = const.tile([128, 128], F32, tag="ones_sq_f",
                                   name="ones_sq_f")
            nc.vector.memset(ones_sq_f, 1.0)
            ones_sq = const.tile([128, 128], F32R, tag="ones_sq", name="ones_sq")
            nc.vector.tensor_copy(ones_sq, ones_sq_f)

            # ---- qT production from x columns 0:half ----
            wo = []
            xqt = []
            bt = []
            qcw = SW  # small first blocks: compute starts after ~0.75MB of DMA
            x_qch = []  # 512-wide x chunks over cols 0:half, reused by jb 0/1
            for qch in range(half // qcw):
                xch = [xchunk(cc, qch * qcw, qcw) for cc in range(2)]
                x_qch.append(xch)
                for d in range(4):
                    ps = pp.tile([128, SW], F32, tag="sim", bufs=3, name="ps_q")
                    for cc in range(2):
                        mm(ps, wqq[cc][:, d * 128:(d + 1) * 128],
                           xch[cc][:, 0:SW],
                           start=(cc == 0), stop=(cc == 1))
                    nc.scalar.copy(qT[d][:, qch * SW:(qch + 1) * SW], ps)
            # final-phase constants, off the startup critical path
            # (vector-queue DMAs so the sync queue stays free for x chunks)
            for d in range(4):
                t = const.tile([128, C], F32R, tag=f"wo{d}", name=f"wo{d}")
                nc.scalar.dma_start(t, woutT[d * 128:(d + 1) * 128, :])
                wo.append(t)
            for cc in range(2):
                t = const.tile([128, half], F32, tag=f"xq{cc}", name=f"xq{cc}")
                nc.scalar.dma_start(t, xq_f[cc * 128:(cc + 1) * 128, :])
                xqt.append(t)
            for cc in range(2):
                t = const.tile([128, 1], F32, tag=f"b{cc}", name=f"b{cc}")
                nc.scalar.dma_start(t, bout[cc])
                bt.append(t)

            # residual-with-bias: xqt <- xqt + b
            for cc in range(2):
                nc.vector.tensor_scalar_add(xqt[cc], xqt[cc], bt[cc])

            # ---- attention over j superblocks ----
            deferred = []  # denominator work deferred into later PE streams
            for jb in range(njb):
                if jb < 2:
                    # cols jb*jbw : (jb+1)*jbw are the query half: reuse the
                    # resident q-phase chunks instead of re-reading x
                    nq = jbw // qcw

                    def xap(cc, start, width, jb=jb, nq=nq):
                        col = jb * jbw + start
                        return x_qch[col // qcw][cc][:, col % qcw:
                                                     col % qcw + width]
                else:
                    xjb = [xchunk(cc, jb * jbw, jbw) for cc in range(2)]

                    def xap(cc, start, width, xjb=xjb):
                        return xjb[cc][:, start:start + width]
                # kT for this superblock: [512, jbw]
                kt = [stream.tile([128, jbw], F32R, tag=f"kt{d}", bufs=1,
                                  name=f"kt{d}") for d in range(4)]
                for d in range(4):
                    for nb in range(jbw // SW):
                        ps = pp.tile([128, SW], F32, tag="sim", bufs=3, name="ps_k")
                        for cc in range(2):
                            mm(ps, wqk[cc][:, d * 128:(d + 1) * 128],
                               xap(cc, nb * SW, SW),
                               start=(cc == 0), stop=(cc == 1))
                        nc.scalar.copy(kt[d][:, nb * SW:(nb + 1) * SW], ps)
                # v for this superblock: [jbw, 512] (token rows on partitions),
                # stored fp8e4 in token-pair planes for DoubleRow po matmuls
                vtp = []
                for t2 in range(jbw // 256):
                    t = stream.tile([128, 2, INNER], F8E4, tag=f"vt{t2}", bufs=1,
                                    name=f"vt{t2}")
                    vtp.append(t)
                for nj in range(jbw // 128):
                    ps = pp.tile([128, INNER], F32, tag="sim", bufs=3, name="ps_v")
                    for cc in range(2):
                        mm(ps, xap(cc, nj * 128, 128),
                           wqv[cc][:, :],
                           start=(cc == 0), stop=(cc == 1))
                    nc.scalar.copy(vtp[nj // 2][:, nj % 2, :], ps)

                for fn in deferred:
                    fn()
                deferred.clear()

                nj8 = jbw // 128
                npair = nj8 // 2
                for s in range(nsl):
                    sl = slice(s * SW, (s + 1) * SW)
                    last_jb = jb == njb - 1
                    po = [pp.tile([128, SW], F32, tag=f"po{d}", bufs=1,
                                  name=f"po{d}") for d in range(4)]
                    if not last_jb:
                        pl = pp.tile([1, SW], F32, tag="aux", bufs=1, name="pl")
                    else:
                        pl = None
                        # l through jb 0..2 in f32r, off the critical path
                        l_rs = work.tile([1, SW], F32R, tag="l_rs", bufs=2,
                                         name="l_rs")
                        nc.scalar.copy(l_rs, l_sb[:, sl])
                    ptp = []

                    sums = []  # binary tree of pT pair-sums (DVE)

                    def tree_add(t):
                        sums.append([t, 0])
                        while len(sums) >= 2 and sums[-1][1] == sums[-2][1]:
                            a, lv = sums.pop()
                            b, _ = sums.pop()
                            t2 = work.tile([128, SW], F32R, tag="pt2", bufs=4,
                                           name="pt2")
                            nc.vector.tensor_add(t2, b, a)
                            sums.append([t2, lv + 1])

                    def l_update(jb=jb, sl=sl, pl=pl):
                        if jb == 0:
                            nc.vector.tensor_copy(l_sb[:, sl], pl)
                        else:
                            nc.vector.tensor_add(l_sb[:, sl], l_sb[:, sl], pl)

                    def drain_pair(p):
                        # outT + denominator work for token-pair p (emitted a
                        # pair late so the PE never waits on the exp). po is a
                        # DoubleRow fp8 matmul: 256 tokens contracted per mm.
                        for d in range(4):
                            mm(po[d], vtp[p][:, :, d * 128:(d + 1) * 128],
                               ptp[p][:, :, :],
                               start=(p == 0), stop=(p == npair - 1),
                               perf_mode=DR)
                        # tree-sum the pair sums on the DVE; the ones-matmul +
                        # l update (non-last jb) or the finalize (last jb) are
                        # deferred into a later PE stream so the PE never
                        # waits on the adder tree
                        pt2 = work.tile([128, SW], F32R, tag="pt2",
                                        bufs=4, name="pt2")
                        nc.vector.tensor_add(pt2, ptp[p][:, 0, :],
                                             ptp[p][:, 1, :])
                        tree_add(pt2)
                        if p == npair - 1 and not last_jb:
                            assert len(sums) == 1
                            pt8 = sums[0][0]

                            def flush(pl=pl, pt8=pt8, upd=l_update):
                                mm(pl, ones_col, pt8, start=True, stop=True)
                                upd()
                            deferred.append(flush)

                    for j8 in range(nj8):
                        ps = pp.tile([128, SW], F32, tag="sim", bufs=3, name="ps_s")
                        for d in range(4):
                            mm(ps, kt[d][:, j8 * 128:(j8 + 1) * 128], qT[d][:, sl],
                               start=(d == 0), stop=(d == 3))
                        p, parity = divmod(j8, 2)
                        if parity == 0:
                            t = work.tile([128, 2, SW], F8E5, tag="pt", bufs=4,
                                          name="pt")
                            ptp.append(t)
                        nc.scalar.activation(ptp[p][:, parity, :], ps, Exp,
                                             scale=SCALE, bias=nshift)
                        if parity == 1 and p > 0:
                            drain_pair(p - 1)
                        if j8 == 2:
                            for fn in deferred:
                                fn()
                            deferred.clear()
                    drain_pair(npair - 1)
                    if not last_jb:
                        for d in range(4):
                            if jb == 0:
                                nc.vector.tensor_copy(ot[d][:, sl], po[d])
                            else:
                                nc.vector.tensor_add(ot[d][:, sl], ot[d][:, sl],
                                                     po[d])
                    else:
                        # ---- finalize slice s: normalize + project + out ----
                        assert len(sums) == 1
                        pt8 = sums[0][0]
                        # otr = ot (jb 0..2) + po (jb 3), fused accumulate +
                        # f32r convert, emitted now so the DVE adds overlap the
                        # next slice's sim stream
                        otr = [work.tile([128, SW], F32R, tag=f"otr{d}", bufs=1,
                                         name=f"otr{d}") for d in range(4)]
                        for d in range(4):
                            nc.vector.tensor_add(otr[d], ot[d][:, sl], po[d])

                        def finalize(sl=sl, pt8=pt8, l_rs=l_rs, otr=otr):
                            # total l broadcast to 128 partitions in one psum
                            # accumulation: colsum(pt8) via all-ones stationary
                            # + broadcast of l_sb (jb 0..2) via ones_row
                            pb = pp.tile([128, SW], F32, tag="aux", bufs=1,
                                         name="pb")
                            mm(pb, ones_sq, pt8, start=True, stop=False)
                            mm(pb, ones_row, l_rs, start=False, stop=True)
                            bc = work.tile([128, SW], F32, tag="bc", bufs=2,
                                           name="bc")
                            rsc = work.tile([128, SW], F32, tag="rsc", bufs=2,
                                            name="rsc")
                            nc.vector.reciprocal_approx_accurate(bc, pb, rsc)
                            for cc in range(2):
                                pf = pp.tile([128, SW], F32, tag="sim", bufs=3,
                                             name="pf")
                                for d in range(4):
                                    mm(pf, wo[d][:, cc * 128:(cc + 1) * 128],
                                       otr[d], start=(d == 0), stop=(d == 3))
                                fo = work.tile([128, SW], F32, tag="fo", bufs=2,
                                               name="fo")
                                nc.vector.tensor_mul(fo, pf, bc)
                                nc.vector.tensor_add(fo, fo, xqt[cc][:, sl])
                                nc.sync.dma_start(out[cc * 128:(cc + 1) * 128, sl],
                                                  fo)
                        if s == nsl - 1:
                            finalize()
                        else:
                            deferred.append(finalize)

    nc.finalize()
    return nc


_NC_CACHE = None


def _get_nc():
    global _NC_CACHE
    if _NC_CACHE is None:
        _NC_CACHE = build_nc()
    return _NC_CACHE


def _round_f32r(a):
    """fp32 -> float32r rounding (round-half-even on the low 12 mantissa
    bits), matching the hardware's fp32_to_fp32r conversion."""
    bits = np.ascontiguousarray(a, dtype=np.float32).view(np.uint32)
    rem = bits & np.uint32(0xFFF)
    base = bits & np.uint32(0xFFFFF000)
    up = (rem > 0x800) | ((rem == 0x800) & (((bits >> np.uint32(12)) & np.uint32(1)) == 1))
    return (base + np.where(up, np.uint32(0x1000), np.uint32(0))).view(np.float32)


def prepare_in_maps(x, w_qkv, w_out, b_out):
    x = np.asarray(x, dtype=np.float32)
    w_qkv = np.asarray(w_qkv, dtype=np.float32)
    w_out = np.asarray(w_out, dtype=np.float32)
    b_out = np.asarray(b_out, dtype=np.float32)

    xr = x.reshape(B, C, N)
    wqkvT = _round_f32r(np.ascontiguousarray(w_qkv.T))   # [C, 1536]
    woutT = _round_f32r(np.ascontiguousarray(w_out.T))   # [512, C]
    bout = np.ascontiguousarray(b_out.reshape(2, 128, 1))

    in_maps = []
    for c in range(NCORES):
        b, h = divmod(c, 2)
        if h == 0:
            x_rot = xr[b]
        else:  # rotate so this core's query half sits in columns 0:HALF
            x_rot = np.concatenate([xr[b][:, HALF:], xr[b][:, :HALF]], axis=1)
        in_maps.append({
            "x_r": _round_f32r(x_rot),
            "xq_f": np.ascontiguousarray(x_rot[:, :HALF]),
            "wqkvT": wqkvT,
            "woutT": woutT,
            "bout": bout,
        })
    return in_maps


def postprocess(results):
    outs = [results[c]["out"] for c in range(NCORES)]
    full = np.stack([np.concatenate([outs[2 * b], outs[2 * b + 1]], axis=1)
                     for b in range(B)])               # [B, C, N]
    return full.reshape(B, C, 64, 64).astype(np.float32)


def kernel(x, w_qkv, w_out, b_out):
    in_maps = prepare_in_maps(x, w_qkv, w_out, b_out)
    res = run_bass_kernel_spmd(_get_nc(), in_maps, core_ids=list(range(NCORES)))
    return postprocess(res.results)

